# revision 15
# baseline (speedup 1.0000x reference)
"""Trainium2 Bass kernel for the MultiHeadAttention block (B=2, S=2048, D=1024, H=16).

Computation (matches the reference nn.Module):
    qkv  = relu(q @ W_qkv + b_qkv)                      # fused QKV projection
    Q,K,V = split(qkv); per-head attention with scores / sqrt(D)
    attn = softmax(QK^T / 32)                           # also an output, [H*B, S, S]
    out  = (attn @ V) @ W_proj + b_proj
    y    = layernorm(out + q) * gamma + beta

Sharding: 8 cores, head-parallel. Core c owns heads {2c, 2c+1} for both
batches (i.e. rows [4c, 4c+4) of the head-major [H*B, S, S] attention
output). The output projection is a partial sum over each core's 128 rows
of W_proj; a ReduceScatter sums the partials and hands core c the
s-rows [c*256, (c+1)*256) of each batch for the residual+LayerNorm tail.

On-core dataflow (per (head, batch) pair):
    scores S[q,k] via PE (fp32r), exp on ACT with fused row-sum (accum_out),
    normalize in place on DVE, fp32 attn chunk DMA'd out, bf16 copy
    DMA-transposed (xbar) into PT[k,q], attn @ V on PE in bf16,
    projection partials on PE (fp32r).
"""
import numpy as np
import concourse.bass as bass
import concourse.bacc as bacc
import concourse.tile as tile
from concourse import mybir
from concourse.bass_utils import run_bass_kernel_spmd
from concourse.masks import make_identity

B, S, D, H = 2, 2048, 1024, 16
DK = D // H            # 64
NCORES = 8
HPC = H // NCORES      # heads per core = 2
TEMPER = float(np.sqrt(D))  # 32.0
EPS = 1e-6
BS = B * S             # 4096
SLC = S // NCORES      # 256 s-rows per core per batch after reduce-scatter

F32 = mybir.dt.float32
F32R = mybir.dt.float32r
BF16 = mybir.dt.bfloat16
AF = mybir.ActivationFunctionType
ALU = mybir.AluOpType

_COMPILED = None


def _build():
    nc = bacc.Bacc("TRN2", target_bir_lowering=False, debug=False, num_devices=NCORES)

    # ---------------- I/O ----------------
    qT = nc.dram_tensor("qT", [D, BS], F32, kind="ExternalInput")       # q^T, cols b*2048+s
    w3 = nc.dram_tensor("w3", [D, 3 * 128], F32, kind="ExternalInput")  # per-core [Q|K|V] cols
    b3 = nc.dram_tensor("b3", [3 * 128], F32, kind="ExternalInput")
    wp = nc.dram_tensor("wp", [128, D], F32, kind="ExternalInput")      # W_proj rows slice
    bp = nc.dram_tensor("bp", [D], F32, kind="ExternalInput")
    gam = nc.dram_tensor("gam", [D], F32, kind="ExternalInput")
    bet = nc.dram_tensor("bet", [D], F32, kind="ExternalInput")
    qres = nc.dram_tensor("qres", [B, D, SLC], F32, kind="ExternalInput")  # qT residual slice

    attn_o = nc.dram_tensor("attn", [2 * B, S, S], F32, kind="ExternalOutput")
    y_o = nc.dram_tensor("y", [B, SLC, D], F32, kind="ExternalOutput")
    import os
    dbg_o = None
    if os.environ.get("K_DEBUG"):
        dbg_o = nc.dram_tensor("dbg", [128, 3, BS], F32, kind="ExternalOutput")
    dbg_s = None
    if os.environ.get("K_DEBUG"):
        dbg_s = nc.dram_tensor("dbgs", [16, 128, S], F32, kind="ExternalOutput")

    with tile.TileContext(nc) as tc:
        _body(nc, tc, qT, w3, b3, wp, bp, gam, bet, qres, attn_o, y_o, dbg_o, dbg_s)
    nc.compile()
    return nc


def _body(nc, tc, qT, w3, b3, wp, bp, gam, bet, qres, attn_o, y_o, dbg_o=None, dbg_s=None):
    ctx_pools = []

    def bcast_row(dram_t, n):
        """AP broadcasting a length-n DRAM row vector across 128 partitions."""
        a = dram_t.ap()
        return bass.AP(tensor=a.tensor, offset=a.offset, ap=[[0, 128], [1, n]])

    with tc.tile_pool(name="const", bufs=1) as const, \
         tc.tile_pool(name="big", bufs=1) as big, \
         tc.tile_pool(name="drp", bufs=1, space="DRAM") as drp, \
         tc.tile_pool(name="psS", bufs=3, space="PSUM") as psS, \
         tc.tile_pool(name="psO", bufs=2, space="PSUM") as psO:

        # ---- constants ----
        w3_sb = const.tile([128, 8, 384], F32R)
        nc.sync.dma_start(
            out=w3_sb,
            in_=w3.ap().rearrange("(dc p) c -> p dc c", p=128).bitcast(F32R))
        wp_sb = const.tile([128, 8, 128], F32R)
        nc.sync.dma_start(
            out=wp_sb, in_=wp.ap().rearrange("p (m c) -> p m c", c=128).bitcast(F32R))
        b3_sb = const.tile([128, 3], F32)
        nc.sync.dma_start(out=b3_sb, in_=b3.ap().rearrange("(m p) -> p m", p=128))
        bp_sb = const.tile([128, 8], F32)
        nc.sync.dma_start(out=bp_sb, in_=bp.ap().rearrange("(m p) -> p m", p=128))
        gam_sb = const.tile([128, D], F32)
        nc.sync.dma_start(out=gam_sb, in_=bcast_row(gam, D))
        bet_sb = const.tile([128, D], F32)
        nc.sync.dma_start(out=bet_sb, in_=bcast_row(bet, D))
        id_bf = const.tile([128, 128], BF16)
        make_identity(nc, id_bf)
        id_f32 = const.tile([128, 128], F32)
        make_identity(nc, id_f32)
        eps_sb = const.tile([128, 1], F32)
        nc.vector.memset(eps_sb, EPS)

        # ---- resident big tensors ----
        qkvT = big.tile([128, 3, BS], F32R)       # rows m*128+p of [Q|K|V]^T
        vbf = big.tile([128, 2, 16, 128], BF16)   # V natural, bf16: [k, b, kc, 2*DK]
        oT = big.tile([128, B, S], F32R)          # attn@V transposed: [2*DK, b, q]

        # ================= Phase 1: QKV projection =================
        with nc.named_scope("qkv_proj"), tc.tile_pool(name="qstream", bufs=2) as qstream:
            qTv = qT.ap().rearrange("(dc p) s -> p dc s", p=128).bitcast(F32R)
            for sb in range(4):  # s-blocks of 1024
                qt = qstream.tile([128, 8, 1024], F32R)
                nc.sync.dma_start(out=qt, in_=qTv[:, :, sb * 1024:(sb + 1) * 1024])
                for m in range(3):
                    for sc in range(2):  # 512-wide outputs
                        ps = psS.tile([128, 512], F32, tag="s")
                        for dc in range(8):
                            nc.tensor.matmul(
                                ps,
                                w3_sb[:, dc, m * 128:(m + 1) * 128],
                                qt[:, dc, sc * 512:(sc + 1) * 512],
                                start=(dc == 0), stop=(dc == 7))
                        nc.scalar.activation(
                            out=qkvT[:, m, sb * 1024 + sc * 512:sb * 1024 + (sc + 1) * 512],
                            in_=ps, func=AF.Relu, bias=b3_sb[:, m:m + 1], scale=1.0)

        # ================= Phase 2: V -> natural layout (bf16) =================
        with nc.named_scope("v_prep"), tc.tile_pool(name="vtmp", bufs=1) as vtmp:
            vbfT = vtmp.tile([128, BS], BF16)
            nc.vector.tensor_copy(out=vbfT, in_=qkvT[:, 2, :].bitcast(F32))
            for b in range(B):
                for kc in range(16):
                    for hl in range(HPC):
                        pt = psO.tile([128, 64], BF16, tag="o")
                        nc.tensor.transpose(
                            pt,
                            vbfT[hl * 64:(hl + 1) * 64,
                                 b * S + kc * 128:b * S + (kc + 1) * 128],
                            id_bf[hl * 64:(hl + 1) * 64, hl * 64:(hl + 1) * 64])
                        nc.vector.tensor_copy(
                            out=vbf[:, b, kc, hl * 64:(hl + 1) * 64], in_=pt)

        # ================= Phase 3: attention =================
        with nc.named_scope("attention"), \
             tc.tile_pool(name="pbuf", bufs=3) as pbuf, \
             tc.tile_pool(name="abf", bufs=3) as abf, \
             tc.tile_pool(name="ptp", bufs=2) as ptp, \
             tc.tile_pool(name="dsm", bufs=6) as dsm:
            for pair in range(2 * B):  # local pair j -> (head hl, batch b)
                hl, b = pair // B, pair % B
                qrow = slice(hl * 64, (hl + 1) * 64)
                for qs in range(4):  # q-superblocks of 512
                    ptile = ptp.tile([128, 4, 16, 128], BF16)  # PT[k, qc, kc, q]
                    for qc4 in range(4):
                        qc = qs * 4 + qc4
                        # scores S[q, k] for 128 q rows, in two 1024-wide halves
                        p = pbuf.tile([128, S], F32)
                        dens = dsm.tile([128, 4], F32, tag="den")
                        for half in range(2):
                            ps = psS.tile([128, 1024], F32, tag="s")
                            for j in range(2):
                                nc.tensor.matmul(
                                    ps[:, j * 512:(j + 1) * 512],
                                    qkvT[qrow, 0, b * S + qc * 128:b * S + (qc + 1) * 128],
                                    qkvT[qrow, 1,
                                         b * S + half * 1024 + j * 512:
                                         b * S + half * 1024 + (j + 1) * 512],
                                    start=True, stop=True)
                            nc.scalar.activation(
                                out=p[:, half * 1024:(half + 1) * 1024], in_=ps,
                                func=AF.Exp, scale=1.0 / TEMPER,
                                accum_out=dens[:, half:half + 1])
                            if dbg_s is not None and pair == 0:
                                sdump = dsm.tile([128, 1024], F32, tag="sd")
                                nc.vector.tensor_copy(out=sdump, in_=ps)
                                nc.sync.dma_start(
                                    out=dbg_s.ap()[qc, :, half * 1024:(half + 1) * 1024],
                                    in_=sdump)
                        den = dsm.tile([128, 1], F32, tag="dent")
                        nc.vector.tensor_tensor(
                            out=den, in0=dens[:, 0:1], in1=dens[:, 1:2], op=ALU.add)
                        inv = dsm.tile([128, 1], F32, tag="inv")
                        nc.vector.reciprocal(out=inv, in_=den)
                        # normalize in place, then fp32 out + bf16 for transpose
                        nc.vector.tensor_scalar(
                            out=p, in0=p, scalar1=inv, scalar2=None, op0=ALU.mult)
                        nc.sync.dma_start(
                            out=attn_o.ap()[pair, qc * 128:(qc + 1) * 128, :], in_=p)
                        pb = abf.tile([128, S], BF16)
                        nc.vector.tensor_copy(out=pb, in_=p)
                        nc.sync.dma_start_transpose(out=ptile[:, qc4, :, :], in_=pb)
                    # attn @ V for this q-superblock: O^T[d, q] accumulated over kc
                    po = psO.tile([64, 512], F32, tag="o")
                    for kc in range(16):
                        nc.tensor.matmul(
                            po,
                            vbf[:, b, kc, hl * 64:(hl + 1) * 64],
                            ptile[:, :, kc, :],
                            start=(kc == 0), stop=(kc == 15))
                    nc.scalar.activation(
                        out=oT[hl * 64:(hl + 1) * 64, b, qs * 512:(qs + 1) * 512],
                        in_=po, func=AF.Copy, scale=1.0)

        if dbg_o is not None:
            nc.sync.dma_start(out=dbg_o.ap()[:, :, :], in_=qkvT.bitcast(F32))
        # ================= Phase 4: output projection partials =================
        with nc.named_scope("proj"), tc.tile_pool(name="prj", bufs=3) as prj:
            ypart = drp.tile([NCORES, B, D, SLC], F32, tag="yp")
            yrs = drp.tile([B, D, SLC], F32, tag="yr")
            for b in range(B):
                for m in range(8):
                    for sc in range(4):
                        ps = psS.tile([128, 512], F32, tag="s")
                        nc.tensor.matmul(
                            ps, wp_sb[:, m, :], oT[:, b, sc * 512:(sc + 1) * 512],
                            start=True, stop=True)
                        sb = prj.tile([128, 512], F32)
                        nc.vector.tensor_copy(out=sb, in_=ps)
                        dst = ypart[sc * 2:sc * 2 + 2, b,
                                    m * 128:(m + 1) * 128, :]
                        nc.sync.dma_start(
                            out=dst.rearrange("i d s -> d i s"), in_=sb)
            nc.gpsimd.collective_compute(
                "ReduceScatter", ALU.add,
                replica_groups=[list(range(NCORES))],
                ins=[ypart.opt()], outs=[yrs.opt()])

        # ================= Phase 5: residual + LayerNorm =================
        with nc.named_scope("layernorm"), \
             tc.tile_pool(name="ln", bufs=2) as ln, \
             tc.tile_pool(name="lnx", bufs=2) as lnx, \
             tc.tile_pool(name="lns", bufs=4) as lns:
            for b in range(B):
                xT = ln.tile([128, 8, SLC], F32)  # [d, dc, s'] for this batch
                yv = yrs.rearrange("b (dc p) s -> b p dc s", p=128)
                nc.sync.dma_start(out=xT, in_=yv[b])
                qr = ln.tile([128, 8, SLC], F32, tag="qr")
                nc.sync.dma_start(
                    out=qr, in_=qres.ap().rearrange("b (dc p) s -> b p dc s", p=128)[b])
                xs = ln.tile([128, 8, SLC], F32, tag="xs")
                for dc in range(8):
                    # x^T = ypartial + b_proj + residual
                    nc.vector.tensor_scalar(
                        out=xs[:, dc, :], in0=xT[:, dc, :],
                        scalar1=bp_sb[:, dc:dc + 1], scalar2=None, op0=ALU.add)
                    nc.vector.tensor_tensor(
                        out=xs[:, dc, :], in0=xs[:, dc, :], in1=qr[:, dc, :], op=ALU.add)
                for sc in range(SLC // 128):  # transpose to natural [s, d]
                    xn = lnx.tile([128, D], F32)
                    for dc in range(8):
                        pt = psO.tile([128, 128], F32, tag="o")
                        nc.tensor.transpose(
                            pt, xs[:, dc, sc * 128:(sc + 1) * 128], id_f32)
                        nc.vector.tensor_copy(
                            out=xn[:, dc * 128:(dc + 1) * 128], in_=pt)
                    stats = lns.tile([128, 2, 6], F32, tag="st")
                    for g in range(2):
                        nc.vector.bn_stats(out=stats[:, g, :], in_=xn[:, g * 512:(g + 1) * 512])
                    mv = lns.tile([128, 2], F32, tag="mv")
                    nc.vector.bn_aggr(out=mv, in_=stats)
                    sd = lns.tile([128, 1], F32, tag="sd")
                    nc.scalar.activation(out=sd, in_=mv[:, 1:2], func=AF.Sqrt,
                                         bias=eps_sb, scale=1.0)
                    rstd = lns.tile([128, 1], F32, tag="rs")
                    nc.vector.reciprocal(out=rstd, in_=sd)
                    nmean = lns.tile([128, 1], F32, tag="nm")
                    nc.scalar.mul(out=nmean, in_=mv[:, 0:1], mul=-1.0)
                    t = lnx.tile([128, D], F32, tag="t")
                    nc.vector.tensor_scalar(
                        out=t, in0=xn, scalar1=nmean, scalar2=rstd,
                        op0=ALU.add, op1=ALU.mult)
                    nc.vector.tensor_tensor(out=t, in0=t, in1=gam_sb, op=ALU.mult)
                    nc.vector.tensor_tensor(out=t, in0=t, in1=bet_sb, op=ALU.add)
                    nc.sync.dma_start(
                        out=y_o.ap()[b, sc * 128:(sc + 1) * 128, :], in_=t)


def _get_compiled():
    global _COMPILED
    if _COMPILED is None:
        _COMPILED = _build()
    return _COMPILED


def _make_in_maps(q, W_qkv, b_qkv, W_proj, b_proj, gamma, beta):
    q = np.ascontiguousarray(np.asarray(q, dtype=np.float32))
    W_qkv = np.asarray(W_qkv, dtype=np.float32)
    b_qkv = np.asarray(b_qkv, dtype=np.float32)
    W_proj = np.asarray(W_proj, dtype=np.float32)

    qT = np.ascontiguousarray(q.reshape(BS, D).T)  # [D, BS]

    in_maps = []
    for c in range(NCORES):
        cs = slice(c * 128, (c + 1) * 128)
        w3 = np.ascontiguousarray(np.concatenate(
            [W_qkv[:, 0 * D + c * 128:0 * D + (c + 1) * 128],
             W_qkv[:, 1 * D + c * 128:1 * D + (c + 1) * 128],
             W_qkv[:, 2 * D + c * 128:2 * D + (c + 1) * 128]], axis=1))
        b3 = np.ascontiguousarray(np.concatenate(
            [b_qkv[0 * D + c * 128:0 * D + (c + 1) * 128],
             b_qkv[1 * D + c * 128:1 * D + (c + 1) * 128],
             b_qkv[2 * D + c * 128:2 * D + (c + 1) * 128]]))
        qres = np.ascontiguousarray(np.stack(
            [qT[:, b * S + c * SLC:b * S + (c + 1) * SLC] for b in range(B)]))
        in_maps.append({
            "qT": qT, "w3": w3, "b3": b3,
            "wp": np.ascontiguousarray(W_proj[cs, :]),
            "bp": np.ascontiguousarray(np.asarray(b_proj, dtype=np.float32)),
            "gam": np.ascontiguousarray(np.asarray(gamma, dtype=np.float32)),
            "bet": np.ascontiguousarray(np.asarray(beta, dtype=np.float32)),
            "qres": qres,
        })
    return in_maps


def _assemble(results):
    attn = np.concatenate([results[c]["attn"] for c in range(NCORES)], axis=0)
    y = np.empty((B, S, D), dtype=np.float32)
    for c in range(NCORES):
        for b in range(B):
            y[b, c * SLC:(c + 1) * SLC, :] = results[c]["y"][b]
    return (y, attn)


def kernel(q, k, v, attn_mask, W_qkv, b_qkv, W_proj, b_proj, gamma, beta):
    in_maps = _make_in_maps(q, W_qkv, b_qkv, W_proj, b_proj, gamma, beta)
    nc = _get_compiled()
    res = run_bass_kernel_spmd(nc, in_maps, list(range(NCORES))).results
    return _assemble(res)
